# revision 12
# baseline (speedup 1.0000x reference)
"""Trainium2 Bass kernel: batched multi-head cross-attention.

Reference computation (per batch element b of 8, one NeuronCore each):
    K,V from x; Q from y (heads=16, dim=1024, d_head=64, scale=dim**-0.5)
    out = softmax(Q K^T * scale) V  -> concat heads -> @ w_out.T + b_out

Sharding: pure data-parallel on batch (8 batch elements -> 8 cores).
No collectives. All layout transposes are done host-side so the on-device
pipeline needs zero transposes:

  xT, yT           : [dim, n]   (feature-major inputs)
  wqkT             : [dim, 2*dim]  t-major column blocks [wq_t | wk_t]
                     (Q block pre-scaled by softmax scale)
  wvT              : [dim, dim]
  woutT            : [dim, dim]  = w_out.T
  biasb            : [128, dim]  = b_out broadcast over partitions

Device pipeline per core (bf16 matmuls, fp32 PSUM accumulation):
  V[j,f]   = xT.T @ wvT            (c-major streaming: MMs start as soon
                                    as the first x/wv c-tiles land)
  Q^T[f,i] = wqT.T @ yT            (scale folded into wqT on host)
  K^T[f,j] = wkT.T @ xT
  dots^T[j,i] = k_h^T.T @ q_h^T    (K=64; head pair packed into row
                                    strips (0,0)/(64,0) -> concurrent MMs)
  exp on ScalarE psum->sbuf bf16
  [out_h^T; s_h] = [v_h|1].T @ exp  (M=65: row 64 = softmax denominator)
  psum evacuated to sbuf by one DVE copy; the normalize chain
  (s hop -> reciprocal -> partition_broadcast -> mul) runs off-path.
  res[i,g] = O^T.T @ woutT + bias  (natural layout, contiguous DMA out)
"""

from contextlib import ExitStack

import numpy as np
import ml_dtypes

import concourse.bass as bass
import concourse.tile as tile
from concourse import bacc, mybir
from concourse.bass_utils import run_bass_kernel_spmd

DIM = 1024
N = 1024
HEADS = 16
DH = 64  # head dim
SCALE = DIM ** -0.5
P = 128          # partitions
NH = 512         # free-dim half (one PSUM bank of fp32)
BF16 = mybir.dt.bfloat16
F32 = mybir.dt.float32
EXP = mybir.ActivationFunctionType.Exp


def build_attention_nc():
    nc = bacc.Bacc("TRN2", target_bir_lowering=False, debug=False)

    xT_d = nc.dram_tensor("xT", [DIM, N], BF16, kind="ExternalInput")
    yT_d = nc.dram_tensor("yT", [DIM, N], BF16, kind="ExternalInput")
    wqkT_d = nc.dram_tensor("wqkT", [DIM, 2 * DIM], BF16, kind="ExternalInput")
    wvT_d = nc.dram_tensor("wvT", [DIM, DIM], BF16, kind="ExternalInput")
    woutT_d = nc.dram_tensor("woutT", [DIM, DIM], BF16, kind="ExternalInput")
    biasb_d = nc.dram_tensor("biasb", [P, DIM], F32, kind="ExternalInput")
    out_d = nc.dram_tensor("out", [N, DIM], F32, kind="ExternalOutput")

    CT = DIM // P   # 8 contraction tiles
    FT = DIM // P   # 8 feature tiles (per Q/K block) == head pairs
    JT = N // P     # 8 key-token tiles
    IT = N // P     # 8 query-token tiles

    with TileBuild(nc) as b:
        tc = b.tc
        ctx = b.ctx

        wqk_pool = ctx.enter_context(tc.tile_pool(name="wqk", bufs=32))
        xy_pool = ctx.enter_context(tc.tile_pool(name="xy", bufs=2 * CT))
        qk_pool = ctx.enter_context(tc.tile_pool(name="qk", bufs=6))
        va_pool = ctx.enter_context(tc.tile_pool(name="va", bufs=JT))
        wo_pool = ctx.enter_context(tc.tile_pool(name="wo", bufs=FT))
        ex_pool = ctx.enter_context(tc.tile_pool(name="ex", bufs=22))
        ot_pool = ctx.enter_context(tc.tile_pool(name="ot", bufs=FT))
        sm_pool = ctx.enter_context(tc.tile_pool(name="sm", bufs=3))
        sa_pool = ctx.enter_context(tc.tile_pool(name="sa", bufs=4))
        res_pool = ctx.enter_context(tc.tile_pool(name="res", bufs=2))
        pd_pool = ctx.enter_context(tc.tile_pool(name="pd", bufs=3, space="PSUM"))
        pa_pool = ctx.enter_context(tc.tile_pool(name="pa", bufs=2, space="PSUM"))

        # ---- load inputs: wqk t=0 chunk first, then x + y interleaved
        # (Q/K t=0 projection starts ASAP -> exp stream starts early),
        # then wv, the remaining wqk chunks, wout, bias ----
        wv_ctx = ExitStack()
        wv_pool = wv_ctx.enter_context(tc.tile_pool(name="wv", bufs=CT))
        wv_t, xT_t, yT_t = [], [], []
        wqk_t = {}
        for c in range(CT):
            t = wqk_pool.tile([P, 256], BF16, tag="wqk", name=f"wqk{c}_0")
            nc.sync.dma_start(t[:], wqkT_d[c * P:(c + 1) * P, 0:256])
            wqk_t[(c, 0)] = t
        for c in range(CT):
            t = xy_pool.tile([P, N], BF16, tag="xy", name=f"xt{c}")
            nc.sync.dma_start(t[:], xT_d[c * P:(c + 1) * P, :])
            xT_t.append(t)
            t = xy_pool.tile([P, N], BF16, tag="xy", name=f"yt{c}")
            nc.sync.dma_start(t[:], yT_d[c * P:(c + 1) * P, :])
            yT_t.append(t)
        for c in range(CT):
            t = wv_pool.tile([P, DIM], BF16, tag="wv", name=f"wv{c}")
            nc.sync.dma_start(t[:], wvT_d[c * P:(c + 1) * P, :])
            wv_t.append(t)
        wo_t = []

        def emit_wqk_chunks(t_blk):
            for c in range(CT):
                t = wqk_pool.tile([P, 256], BF16, tag="wqk",
                                  name=f"wqk{c}_{t_blk}")
                nc.sync.dma_start(
                    t[:], wqkT_d[c * P:(c + 1) * P,
                                 t_blk * 256:(t_blk + 1) * 256])
                wqk_t[(c, t_blk)] = t

        def emit_wo_bias():
            bias_t = res_pool.tile([P, DIM], F32, tag="bias", bufs=1)
            nc.sync.dma_start(bias_t[:], biasb_d[:, :])
            for f in range(FT):
                t = wo_pool.tile([P, DIM], BF16, tag="wo", name=f"wo{f}")
                nc.sync.dma_start(t[:], woutT_d[f * P:(f + 1) * P, :])
                wo_t.append(t)
            return bias_t

        OT = [ot_pool.tile([P, N], BF16, tag="ot", name=f"OT{f}") for f in range(FT)]
        VA = [None] * JT

        # ---- V projection chunk: j-major so VA[j] tiles finalize in
        # order while ScalarE chews on the first pairs' exps ----
        def emit_vproj(js):
            for j in js:
                ps = pd_pool.tile([P, N], F32, tag="pd", name=f"psv{j}")
                for n in range(2):
                    for c in range(CT):
                        nc.tensor.matmul(
                            ps[:, n * NH:(n + 1) * NH],
                            lhsT=xT_t[c][:, j * P:(j + 1) * P],
                            rhs=wv_t[c][:, n * NH:(n + 1) * NH],
                            start=(c == 0), stop=(c == CT - 1),
                        )
                va = va_pool.tile([P, HEADS, DH + 1], BF16, tag="va",
                                  name=f"va{j}")
                nc.vector.tensor_copy(
                    va[:, :, 0:DH],
                    ps[:].rearrange("p (h c) -> p h c", c=DH),
                )
                nc.vector.memset(va[:, :, DH:DH + 1], 1.0)
                VA[j] = va

        # ---- Q/K projection for head pair t ----
        def emit_proj(t):
            qt_kt = []
            for which, rhs_t in ((0, yT_t), (1, xT_t)):
                ps = pd_pool.tile([P, N], F32, tag="pd", name=f"psp{which}_{t}")
                for n in range(2):
                    for c in range(CT):
                        nc.tensor.matmul(
                            ps[:, n * NH:(n + 1) * NH],
                            lhsT=wqk_t[(c, t)][:, which * P:
                                               (which + 1) * P],
                            rhs=rhs_t[c][:, n * NH:(n + 1) * NH],
                            start=(c == 0), stop=(c == CT - 1),
                        )
                sb = qk_pool.tile([P, N], BF16, tag="qk", name=f"qk{which}_{t}")
                nc.vector.tensor_copy(sb[:, 0:NH], ps[:, 0:NH])
                nc.vector.tensor_copy(sb[:, NH:N], ps[:, NH:N])
                qt_kt.append(sb)
            return qt_kt

        # ---- dots + exp for head pair t: row strips (0,0)/(64,0)
        # interleaved so the two K=64 matmuls run concurrently ----
        def emit_dots_exp(t, QTt, KTt):
            ex_t = {0: [], 1: []}
            for j in range(JT):
                ps_p = []
                for par in range(2):
                    ps = pd_pool.tile([P, N], F32, tag="pd",
                                      name=f"psd{t}_{par}_{j}")
                    ps_p.append(ps)
                for n in range(2):
                    for par in range(2):
                        pb = par * DH
                        nc.tensor.matmul(
                            ps_p[par][:, n * NH:(n + 1) * NH],
                            lhsT=KTt[pb:pb + DH, j * P:(j + 1) * P],
                            rhs=QTt[pb:pb + DH, n * NH:(n + 1) * NH],
                            start=True, stop=True,
                            tile_position=(pb, 0),
                        )
                for par in range(2):
                    h = 2 * t + par
                    ex = ex_pool.tile([P, N], BF16, tag="ex", name=f"ex{h}_{j}")
                    nc.scalar.activation(ex[:], ps_p[par][:], EXP)
                    ex_t[par].append(ex)
            return ex_t

        # ---- attn@V + softmax-normalize for head pair t ----
        def emit_attnv(t, ex_t):
            for par in range(2):
                h = 2 * t + par
                for n in range(2):
                    acc = pa_pool.tile([P, NH], F32, tag="pa", name=f"acc{h}_{n}")
                    for j in range(JT):
                        nc.tensor.matmul(
                            acc[0:DH + 1, :],
                            lhsT=VA[j][:, h, :],
                            rhs=ex_t[par][j][:, n * NH:(n + 1) * NH],
                            start=(j == 0), stop=(j == JT - 1),
                        )
                    # Evacuate psum with one DVE copy; the normalize chain
                    # below runs on the sbuf copy, off the PE critical path.
                    sb_acc = sa_pool.tile([DH + 1, NH], F32, tag="sa",
                                          name=f"sa{h}_{n}")
                    nc.vector.tensor_copy(sb_acc[:], acc[0:DH + 1, :])
                    # r = 1/s; HW custom ops only honor partition base 0, so
                    # hop s: sbuf[64] -> DMA -> sbuf[0].
                    r_sb = sm_pool.tile([1, 2 * NH], F32, tag="rs",
                                        name=f"rs{h}_{n}", bufs=2)
                    nc.sync.dma_start(out=r_sb[0:1, 0:NH],
                                      in_=sb_acc[DH:DH + 1, :])
                    nc.vector.reciprocal_approx_fast(
                        out=r_sb[0:1, NH:2 * NH], in_=r_sb[0:1, 0:NH])
                    rb = sm_pool.tile([DH, NH], F32, tag="rb", name=f"rb{h}_{n}")
                    nc.gpsimd.partition_broadcast(rb[:], r_sb[0:1, NH:2 * NH])
                    if par == 0:
                        nc.vector.tensor_mul(
                            OT[t][0:DH, n * NH:(n + 1) * NH],
                            sb_acc[0:DH, :], rb[:])
                    else:
                        om = sm_pool.tile([DH, NH], BF16, tag="om",
                                          name=f"om{h}_{n}")
                        nc.vector.tensor_mul(om[:], sb_acc[0:DH, :], rb[:])
                        nc.sync.dma_start(
                            out=OT[t][DH:P, n * NH:(n + 1) * NH], in_=om[:])

        # ---- output projection, split in two f-halves; part 1 runs
        # under the exp stream once OT[0..3] are final ----
        o1 = []

        def emit_outproj_part1():
            for i in range(IT):
                ps = pd_pool.tile([P, N], F32, tag="pd", name=f"psf1_{i}")
                for n in range(2):
                    for f in range(4):
                        nc.tensor.matmul(
                            ps[:, n * NH:(n + 1) * NH],
                            lhsT=OT[f][:, i * P:(i + 1) * P],
                            rhs=wo_t[f][:, n * NH:(n + 1) * NH],
                            start=(f == 0), stop=(f == 3),
                        )
                o1i = o1_pool.tile([P, N], BF16, tag="o1", name=f"o1_{i}")
                nc.vector.tensor_add(o1i[:], ps[:], bias_t[:])
                o1.append(o1i)

        def emit_outproj_part2():
            for i in range(IT):
                ps = pd_pool.tile([P, N], F32, tag="pd", name=f"psf2_{i}")
                for n in range(2):
                    for f in range(4, FT):
                        nc.tensor.matmul(
                            ps[:, n * NH:(n + 1) * NH],
                            lhsT=OT[f][:, i * P:(i + 1) * P],
                            rhs=wo_t[f][:, n * NH:(n + 1) * NH],
                            start=(f == 4), stop=(f == FT - 1),
                        )
                res = res_pool.tile([P, DIM], F32, tag="res", name=f"res{i}")
                nc.vector.tensor_add(res[:], ps[:], o1[i][:])
                nc.sync.dma_start(out=out_d[i * P:(i + 1) * P, :], in_=res[:])

        # ---- emission schedule: QK t=0 first (exp starts ~18us), V proj
        # chunks under the first pairs' exp, dots/exp pipelined one pair
        # ahead of attnV, out-proj part 1 at the halfway point ----
        proj_bufs = {0: emit_proj(0)}
        ex_bufs = {0: emit_dots_exp(0, *proj_bufs.pop(0))}
        emit_wqk_chunks(1)
        emit_wqk_chunks(2)
        emit_vproj([0, 1, 2])
        proj_bufs[1] = emit_proj(1)
        emit_wqk_chunks(3)
        emit_vproj([3, 4, 5])
        proj_bufs[2] = emit_proj(2)
        bias_t = emit_wo_bias()
        emit_wqk_chunks(4)
        emit_vproj([6, 7])
        wv_ctx.close()
        o1_pool = ctx.enter_context(tc.tile_pool(name="o1", bufs=IT))
        for t in range(FT):
            if t + 5 < FT:
                emit_wqk_chunks(t + 5)
            if t + 3 < FT:
                proj_bufs[t + 3] = emit_proj(t + 3)
            if t + 1 < FT:
                ex_bufs[t + 1] = emit_dots_exp(t + 1, *proj_bufs.pop(t + 1))
            emit_attnv(t, ex_bufs.pop(t))
            if t == 3:
                emit_outproj_part1()
        emit_outproj_part2()

    nc.compile()
    return nc


class TileBuild:
    """TileContext + ExitStack pools in one with-block."""

    def __init__(self, nc):
        self.nc = nc
        self.ctx = ExitStack()
        self._tc_cm = tile.TileContext(nc)

    def __enter__(self):
        self.tc = self._tc_cm.__enter__()
        self.ctx.__enter__()
        return self

    def __exit__(self, *exc):
        self.ctx.__exit__(*exc)
        return self._tc_cm.__exit__(*exc)


_NC_CACHE = None


def _get_nc():
    global _NC_CACHE
    if _NC_CACHE is None:
        _NC_CACHE = build_attention_nc()
    return _NC_CACHE


def prepare_inputs(x, y, w_qkv, w_out, b_out):
    bf16 = ml_dtypes.bfloat16
    xT = np.ascontiguousarray(np.transpose(x, (0, 2, 1))).astype(bf16)
    yT = np.ascontiguousarray(np.transpose(y, (0, 2, 1))).astype(bf16)
    wq = np.array(w_qkv, dtype=np.float32, copy=True)
    wq[0:DIM, :] *= SCALE  # fold softmax scale into the Q projection
    wqkvT = np.ascontiguousarray(wq.T)
    # t-major column blocks: [wq_t | wk_t] for t = 0..7
    blocks = []
    for t in range(DIM // P):
        blocks.append(wqkvT[:, t * P:(t + 1) * P])
        blocks.append(wqkvT[:, DIM + t * P:DIM + (t + 1) * P])
    wqkT = np.ascontiguousarray(np.concatenate(blocks, axis=1)).astype(bf16)
    wvT = np.ascontiguousarray(wqkvT[:, 2 * DIM:3 * DIM]).astype(bf16)
    woutT = np.ascontiguousarray(np.array(w_out, dtype=np.float32).T).astype(bf16)
    biasb = np.ascontiguousarray(
        np.broadcast_to(np.array(b_out, dtype=np.float32), (P, DIM)))
    in_maps = []
    for i in range(x.shape[0]):
        in_maps.append({
            "xT": np.ascontiguousarray(xT[i]),
            "yT": np.ascontiguousarray(yT[i]),
            "wqkT": wqkT,
            "wvT": wvT,
            "woutT": woutT,
            "biasb": biasb,
        })
    return in_maps


def kernel(x, y, w_qkv, w_out, b_out, trace=False):
    nc = _get_nc()
    in_maps = prepare_inputs(x, y, w_qkv, w_out, b_out)
    r = run_bass_kernel_spmd(nc, in_maps, core_ids=list(range(len(in_maps))),
                             trace=trace)
    out = np.stack([r.results[i]["out"] for i in range(len(in_maps))])
    if trace:
        kernel.last_results = r
    return out.astype(np.float32)
